# revision 1
# baseline (speedup 1.0000x reference)
"""Multi-head causal self-attention (B=4, S=2048, D=1024, H=16) on 8 NeuronCores.

Sharding: core c handles batch b=c//2 and heads [8*(c%2), 8*(c%2)+8) (tensor
parallel over heads x data parallel over batch). Each core computes its 8
heads' Q/K/V projections, causal attention, and a partial O-projection
(contracting only its 512 ctx dims). Host sums the two partial outputs per
batch.

Kernel math (per core), all matmuls in float32r (TF32-like, full PE rate):
  QT/KT per head-pair:  QT[2*64, S] = wq_pair.T @ xT        (dh on partitions)
  V natural:            V[S, 8*64]  = x @ wv.T, stored per k-tile with an
                        appended ones-column -> softmax denominators fall out
                        of the PV matmul as one extra output row.
  scores (transposed):  ST[k,q] = KT_j.T @ QT  -> exp on ScalarE (PSUM->SBUF,
                        scale=1/8 folded in). No max-subtraction: |scores|<~3.
  causal: per k-tile j and 512-chunk c, only cols >= 128*(j-4c) are valid;
          exp covers only the valid region; 128x128 triangular mask (gpsimd)
          on the diagonal tile.
  PV:                   ctxT[65, q] += V_aug_j.T @ PT_j  (row 64 = denom)
  normalize (off critical path): evacuate ctx+denom rows to SBUF (frees the
          PSUM bank), reciprocal_approx_fast on the denom row, gpsimd
          partition-broadcast (full 128 partitions -- base-64 writes are
          broken on HW), one in-place DVE multiply.
  O-projection:         out[s, D] = sum_ct ctxT_ct.T @ woT_ct, evacuated to
          full [128, D] rows -> single contiguous 512KB DMA per s-tile.
"""
import sys
for _p in ('/opt/trn_rl_repo', '/root/.axon_site/_ro/trn_rl_repo'):
    if _p not in sys.path:
        sys.path.insert(0, _p)

import numpy as np

B, S, D, H = 4, 2048, 1024, 16
DH = 64
N_CORES = 8
HL = H // 2           # local heads per core
DL = HL * DH          # local ctx dims per core


def build_nc(s=S, d=D, hl=HL, n_cores=N_CORES, reps=1):
    """Build the per-core Bass program (shapes parameterizable for sim tests)."""
    import concourse.bacc as bacc
    import concourse.mybir as mybir
    import concourse.tile as tile

    DT = mybir.dt
    F32 = DT.float32
    F32R = DT.float32r
    AFT = mybir.ActivationFunctionType

    dl = hl * DH
    n_kt = s // 128       # k/s tiles
    n_ch = s // 512       # 512-wide q chunks
    n_dt = d // 128       # d_model tiles
    n_oc = d // 512       # output d chunks
    pairs = hl // 2

    nc = bacc.Bacc("TRN2", target_bir_lowering=False, debug=False,
                   num_devices=n_cores)
    xT = nc.declare_dram_parameter("xT", [d, s], F32R, isOutput=False)
    wqT = nc.declare_dram_parameter("wqT", [d, dl], F32R, isOutput=False)
    wkT = nc.declare_dram_parameter("wkT", [d, dl], F32R, isOutput=False)
    wvT = nc.declare_dram_parameter("wvT", [d, dl], F32R, isOutput=False)
    woT = nc.declare_dram_parameter("woT", [dl, d], F32R, isOutput=False)
    tri = nc.declare_dram_parameter("tri", [128, 128], F32R, isOutput=False)
    out = nc.declare_dram_parameter("out", [s, d], F32, isOutput=True)

    with tile.TileContext(nc) as tc:
        with tc.tile_pool(name="persist", bufs=1) as pp, \
             tc.tile_pool(name="stream", bufs=1) as sp, \
             tc.tile_pool(name="psum", bufs=1, space="PSUM") as ps:

            # ---- resident tensors ----
            xt = pp.tile([128, n_dt, s], F32R, name="xt")              # X^T
            v_sb = pp.tile([128, n_kt, hl, DH + 1], F32R, name="v_sb")  # V + ones col
            ctx_all = pp.tile([128, pairs, s], F32R, name="ctx_all")   # normalized ctx^T
            tri_sb = pp.tile([128, 128], F32R, name="tri_sb")

            nc.gpsimd.dma_start(out=tri_sb, in_=tri[:, :])
          
            for _rep in range(reps):
              R = f"{_rep}_" if reps > 1 else ""

            # interleave (wv[t], xt[t]) across both HWDGE queues so the
            # t-accumulation can pace with DMA arrivals
              # xt lands in col-chunk-major order: V k-tiles 4c..4c+3 and Q/K
              # chunk c become computable after chunk c arrives. wv[t] interleaves
              # with chunk 0 so the first V group paces with arrivals.
              wv_sb = sp.tile([128, n_dt, dl], F32R, name=f"{R}wv_sb", tag="wvo")
              for c in range(n_ch):
                  for t in range(n_dt):
                      eng = nc.sync if t % 2 == 0 else nc.scalar
                      if c == 0:
                          eng.dma_start(out=wv_sb[:, t, :],
                                        in_=wvT[128 * t:128 * (t + 1), :])
                      eng.dma_start(out=xt[:, t, 512 * c:512 * (c + 1)],
                                    in_=xT[128 * t:128 * (t + 1), 512 * c:512 * (c + 1)])
              nc.gpsimd.memset(v_sb[:, :, :, DH:DH + 1].bitcast(F32), 1.0)
              for kt in range(n_kt):
                  pv = ps.tile([128, dl], F32, name=f"{R}pv_{kt}",
                               tag=("pp", "stA", "stB")[kt % 3], bufs=2)
                  for t in range(n_dt):
                      nc.tensor.matmul(pv[:, :],
                                       xt[:, t, 128 * kt:128 * (kt + 1)],
                                       wv_sb[:, t, :],
                                       start=(t == 0), stop=(t == n_dt - 1))
                  nc.vector.tensor_copy(
                      out=v_sb[:, kt, :, 0:DH],
                      in_=pv.rearrange("p (h e) -> p h e", e=DH))

              # ---- per head-pair: Q/K projection + attention ----
              for p in range(pairs):
                  qt = sp.tile([128, s], F32R, name=f"{R}qt_{p}", tag="qt", bufs=2)
                  kt_sb = sp.tile([128, s], F32R, name=f"{R}kt_{p}", tag="kt", bufs=1)
                  wq_sb = sp.tile([128, n_dt, 128], F32R, name=f"{R}wq_{p}", tag="wq")
                  wk_sb = sp.tile([128, n_dt, 128], F32R, name=f"{R}wk_{p}", tag="wk")
                  wq_r = wqT.rearrange("(t r) m -> r t m", r=128)
                  wk_r = wkT.rearrange("(t r) m -> r t m", r=128)
                  nc.scalar.dma_start(out=wq_sb, in_=wq_r[:, :, 128 * p:128 * (p + 1)])
                  nc.sync.dma_start(out=wk_sb, in_=wk_r[:, :, 128 * p:128 * (p + 1)])

                  # pair 0's projection precedes all attention: borrow the idle
                  # score banks for a deeper accumulation pipeline
                  ptags = ("pp", "stA", "stB") if p == 0 else ("pp",)
                  for c4 in range(n_ch):
                      psq = ps.tile([128, 512], F32, name=f"{R}psq_{p}_{c4}",
                                    tag=ptags[(2 * c4) % len(ptags)], bufs=2)
                      for t in range(n_dt):
                          nc.tensor.matmul(psq[:, :], wq_sb[:, t, :],
                                           xt[:, t, 512 * c4:512 * (c4 + 1)],
                                           start=(t == 0), stop=(t == n_dt - 1))
                      nc.vector.tensor_copy(out=qt[:, 512 * c4:512 * (c4 + 1)], in_=psq)
                      psk = ps.tile([128, 512], F32, name=f"{R}psk_{p}_{c4}",
                                    tag=ptags[(2 * c4 + 1) % len(ptags)], bufs=2)
                      for t in range(n_dt):
                          nc.tensor.matmul(psk[:, :], wk_sb[:, t, :],
                                           xt[:, t, 512 * c4:512 * (c4 + 1)],
                                           start=(t == 0), stop=(t == n_dt - 1))
                      nc.vector.tensor_copy(out=kt_sb[:, 512 * c4:512 * (c4 + 1)], in_=psk)

                  # attention over chunks
                  for c4 in range(n_ch):
                      q0 = 512 * c4
                      ctxA = ps.tile([DH + 1, 512], F32, name=f"{R}cA_{p}_{c4}", tag="ctxA")
                      ctxB = ps.tile([DH + 1, 512], F32, name=f"{R}cB_{p}_{c4}", tag="ctxB")
                      nj = 4 * c4 + 4
                      pending = None
                      for j in range(nj):
                          m = j - 4 * c4
                          n0 = 128 * m if m >= 0 else 0
                          stA = ps.tile([128, 512], F32, name=f"{R}sA_{p}_{c4}_{j}",
                                        tag="stA", bufs=2)
                          stB = ps.tile([128, 512], F32, name=f"{R}sB_{p}_{c4}_{j}",
                                        tag="stB", bufs=2)
                          ks = slice(128 * j, 128 * (j + 1))
                          # f32r matmuls below N=256 run at 1/4 rate: widen the
                          # narrow diagonal score MMs to 256 (exp ignores the
                          # extra columns)
                          n0_mm = min(n0, 512 - 256)
                          qs = slice(q0 + n0_mm, q0 + 512)
                          nc.tensor.matmul(stA[:, n0_mm:512], kt_sb[0:64, ks],
                                           qt[0:64, qs], start=True, stop=True)
                          nc.tensor.matmul(stB[:, n0_mm:512], kt_sb[64:128, ks],
                                           qt[64:128, qs], start=True, stop=True)
                          ptA = sp.tile([128, 512], F32R, name=f"{R}pA_{p}_{c4}_{j}",
                                        tag="ptA", bufs=3)
                          ptB = sp.tile([128, 512], F32R, name=f"{R}pB_{p}_{c4}_{j}",
                                        tag="ptB", bufs=3)
                          for st_, pt_ in ((stA, ptA), (stB, ptB)):
                              nc.scalar.activation(out=pt_[:, n0:512], in_=st_[:, n0:512],
                                                   func=AFT.Exp, scale=0.125)
                              if m >= 0:
                                  nc.gpsimd.tensor_mul(pt_[:, n0:n0 + 128],
                                                       pt_[:, n0:n0 + 128], tri_sb)
                          if pending is not None:
                              _emit_pv(nc, v_sb, ctxA, ctxB, p, pending, nj)
                          pending = (j, ptA, ptB, n0)
                      _emit_pv(nc, v_sb, ctxA, ctxB, p, pending, nj)

                      # evacuate PSUM fast (frees the ctx bank), then
                      # normalize off the critical path: fast reciprocal of the
                      # denom row (125ns), gpsimd partition-broadcast (810ns),
                      # in-place multiply
                      for head, cpsum in ((0, ctxA), (1, ctxB)):
                          r_i = 2 * c4 + head
                          nc.vector.tensor_copy(
                              out=ctx_all[64 * head:64 * head + 64, p,
                                          q0:q0 + 512],
                              in_=cpsum[0:DH, :])
                          dn1 = sp.tile([1, 512], F32, name=f"{R}dn_{p}_{r_i}",
                                        tag="dn", bufs=1)
                          nc.vector.tensor_copy(out=dn1, in_=cpsum[DH:DH + 1, :])
                          rb1 = sp.tile([1, 512], F32, name=f"{R}rc_{p}_{r_i}",
                                        tag="rc", bufs=1)
                          nc.vector.reciprocal_approx_fast(out=rb1, in_=dn1)
                          rb = sp.tile([128, 512], F32, name=f"{R}rb_{p}_{r_i}",
                                       tag="rb", bufs=1)
                          hs = slice(64 * head, 64 * head + 64)
                          nc.gpsimd.partition_broadcast(rb, rb1)
                          cslice = ctx_all[hs, p, 512 * c4:512 * (c4 + 1)]
                          nc.vector.tensor_mul(cslice, cslice.bitcast(F32),
                                               rb[hs, :])

              # ---- O projection (partial: contracts local 512 ctx dims) ----
              wo_sb = sp.tile([128, pairs, d], F32R, name=f"{R}wo_sb", tag="wvo")
              for ct in range(pairs):
                  nc.scalar.dma_start(out=wo_sb[:, ct, :],
                                      in_=woT[128 * ct:128 * (ct + 1), :])
              for st_i in range(n_kt):
                  # full [128, d] output rows -> one fully-contiguous DMA per
                  # s-tile at max bandwidth
                  ot = sp.tile([128, d], F32, name=f"{R}ot_{st_i}",
                               tag="ot", bufs=3)
                  for oc in range(n_oc):
                      i = st_i * n_oc + oc
                      _ptag = ("stA", "stB", "ctxA", "ctxB")[i % 4]
                      pso = ps.tile([128, 512], F32, name=f"{R}po_{st_i}_{oc}",
                                    tag=_ptag, bufs=(2 if i % 4 < 2 else 1))
                      for ct in range(pairs):
                          nc.tensor.matmul(pso[:, :],
                                           ctx_all[:, ct, 128 * st_i:128 * (st_i + 1)],
                                           wo_sb[:, ct, 512 * oc:512 * (oc + 1)],
                                           start=(ct == 0), stop=(ct == pairs - 1))
                      if oc % 2 == 0:
                          nc.vector.tensor_copy(out=ot[:, 512 * oc:512 * (oc + 1)],
                                                in_=pso)
                      else:
                          nc.scalar.copy(out=ot[:, 512 * oc:512 * (oc + 1)],
                                         in_=pso)
                  oeng = nc.sync if st_i % 2 == 0 else nc.scalar
                  oeng.dma_start(out=out[128 * st_i:128 * (st_i + 1), :], in_=ot)

    nc.compile()
    return nc


def _emit_pv(nc, v_sb, ctxA, ctxB, p, pending, nj):
    j, ptA, ptB, n0 = pending
    start = (j == 0)
    stop = (j == nj - 1)
    nc.tensor.matmul(ctxA[:, n0:512], v_sb[:, j, 2 * p, :], ptA[:, n0:512],
                     start=start, stop=stop)
    nc.tensor.matmul(ctxB[:, n0:512], v_sb[:, j, 2 * p + 1, :], ptB[:, n0:512],
                     start=start, stop=stop)


def make_tri():
    k = np.arange(128)[:, None]
    q = np.arange(128)[None, :]
    return (k <= q).astype(np.float32)


def make_sel(n_ch=S // 512):
    sel = np.zeros((2 * n_ch, n_ch, 128), np.float32)
    for c in range(n_ch):
        sel[2 * c, c, 0:DH] = 1.0
        sel[2 * c + 1, c, DH:128] = 1.0
    return sel


def shard_inputs(in_features, q_weight, k_weight, v_weight, o_weight):
    """-> list of 8 per-core input dicts."""
    tri = make_tri()
    maps = []
    for c in range(N_CORES):
        b, g = divmod(c, 2)
        hs = slice(DL * g, DL * (g + 1))   # local head dims in the full D
        maps.append({
            "xT": np.ascontiguousarray(in_features[b].T),
            "wqT": np.ascontiguousarray(q_weight[hs, :].T),
            "wkT": np.ascontiguousarray(k_weight[hs, :].T),
            "wvT": np.ascontiguousarray(v_weight[hs, :].T),
            "woT": np.ascontiguousarray(o_weight[:, hs].T),
            "tri": tri,
        })
    return maps


def gather_output(results):
    """results: list of 8 dicts with 'out' [S, D] partials -> [B, S, D]."""
    return np.stack([results[2 * b]["out"] + results[2 * b + 1]["out"]
                     for b in range(B)])


_nc_cache = {}


def kernel(in_features, q_weight, k_weight, v_weight, o_weight):
    from concourse.bass_utils import run_bass_kernel_spmd
    if "nc" not in _nc_cache:
        _nc_cache["nc"] = build_nc()
    nc = _nc_cache["nc"]
    in_maps = shard_inputs(np.asarray(in_features, dtype=np.float32),
                           np.asarray(q_weight, dtype=np.float32),
                           np.asarray(k_weight, dtype=np.float32),
                           np.asarray(v_weight, dtype=np.float32),
                           np.asarray(o_weight, dtype=np.float32))
    res = run_bass_kernel_spmd(nc, in_maps, core_ids=list(range(N_CORES)))
    return gather_output(res.results)



# revision 3
# speedup vs baseline: 1.0404x; 1.0404x over previous
"""Multi-head causal self-attention (B=4, S=2048, D=1024, H=16) on 8 NeuronCores.

Sharding: core c handles batch b=c//2 and heads [8*(c%2), 8*(c%2)+8). Host sums
the two partial O-projections per batch.

v3 over the baseline:
  - Attention operands in bf16 (qt/kt/v_sb/probs): no f32r small-N penalty on
    diagonal tiles, halved SBUF enabling double-buffered pipelining.
  - Both heads' exp fused into ONE ScalarE activation per (chunk, k-tile):
    scores land in a single [128, 2, 512] PSUM tile (2 banks).
  - Software pipelining with "filler" PE units inside the attention j-loop:
    pair 0's chunks interleave the V + pair-0 Q/K projections (DMA-paced
    start), pairs 0-2 absorb the next pair's Q/K projection, pair 3 absorbs
    the O-projection of already-normalized chunks.
  - All DMA triggers on sync/scalar (HWDGE); tri-mask muls on DVE; O-proj
    evacuations on DVE (ScalarE does only exp).
"""
import sys
for _p in ('/opt/trn_rl_repo', '/root/.axon_site/_ro/trn_rl_repo'):
    if _p not in sys.path:
        sys.path.insert(0, _p)

import numpy as np

B, S, D, H = 4, 2048, 1024, 16
DH = 64
N_CORES = 8
HL = H // 2           # local heads per core
DL = HL * DH          # local ctx dims per core


def build_nc(s=S, d=D, hl=HL, n_cores=N_CORES, reps=1):
    import concourse.bacc as bacc
    import concourse.mybir as mybir
    import concourse.tile as tile

    DT = mybir.dt
    F32 = DT.float32
    F32R = DT.float32r
    BF16 = DT.bfloat16
    AFT = mybir.ActivationFunctionType

    dl = hl * DH
    n_kt = s // 128       # k/s tiles
    n_ch = s // 512       # 512-wide q chunks
    n_dt = d // 128       # d_model tiles
    n_oc = d // 512       # output d chunks
    pairs = hl // 2

    nc = bacc.Bacc("TRN2", target_bir_lowering=False, debug=False,
                   num_devices=n_cores)
    xT = nc.declare_dram_parameter("xT", [d, s], BF16, isOutput=False)
    wqT = nc.declare_dram_parameter("wqT", [d, dl], BF16, isOutput=False)
    wkT = nc.declare_dram_parameter("wkT", [d, dl], BF16, isOutput=False)
    wvT = nc.declare_dram_parameter("wvT", [d, dl], BF16, isOutput=False)
    woT = nc.declare_dram_parameter("woT", [dl, d], BF16, isOutput=False)
    tri = nc.declare_dram_parameter("tri", [128, 128], BF16, isOutput=False)
    out = nc.declare_dram_parameter("out", [s, d], F32, isOutput=True)

    with tile.TileContext(nc) as tc:
        with tc.tile_pool(name="persist", bufs=1) as pp, \
             tc.tile_pool(name="stream", bufs=1) as sp, \
             tc.tile_pool(name="psum", bufs=1, space="PSUM") as ps:

            # ---- resident tensors ----
            xt = pp.tile([128, n_dt, s], BF16, name="xt")               # X^T
            v_sb = pp.tile([128, n_kt, hl, DH + 1], BF16, name="v_sb")  # V + ones
            ctx_all = pp.tile([128, pairs, s], BF16, name="ctx_all")    # ctx^T
            tri_sb = pp.tile([128, 128], BF16, name="tri_sb")

            nc.sync.dma_start(out=tri_sb, in_=tri[:, :])

            for _rep in range(reps):
              R = f"{_rep}_" if reps > 1 else ""

              # xt lands chunk-major across both HWDGE queues (sync+scalar);
              # pair-0 q/k weights ride along right after chunk 0
              wv_sb = sp.tile([128, n_dt, dl], BF16, name=f"{R}wv_sb", tag="wvo")
              wq_r = wqT.rearrange("(t r) m -> r t m", r=128)
              wk_r = wkT.rearrange("(t r) m -> r t m", r=128)

              def load_qk_weights(p):
                  wq_sb = sp.tile([128, n_dt, 128], BF16,
                                  name=f"{R}wq_{p}", tag="wq", bufs=3)
                  wk_sb = sp.tile([128, n_dt, 128], BF16,
                                  name=f"{R}wk_{p}", tag="wk", bufs=3)
                  nc.sync.dma_start(out=wq_sb, in_=wq_r[:, :, 128 * p:128 * (p + 1)])
                  nc.scalar.dma_start(out=wk_sb, in_=wk_r[:, :, 128 * p:128 * (p + 1)])
                  qt = sp.tile([128, s], BF16, name=f"{R}qt_{p}", tag="qt", bufs=3)
                  kt_sb = sp.tile([128, s], BF16, name=f"{R}kt_{p}", tag="kt", bufs=3)
                  return wq_sb, wk_sb, qt, kt_sb

              xT_r = xT.rearrange("(t r) c -> r t c", r=128)
              for c in range(n_ch):
                  if c == 0:
                      # finest granularity where trickle matters (cold start)
                      for t in range(n_dt):
                          eng = nc.sync if t % 2 == 0 else nc.scalar
                          eng.dma_start(out=wv_sb[:, t, :],
                                        in_=wvT[128 * t:128 * (t + 1), :])
                          eng.dma_start(out=xt[:, t, 0:512],
                                        in_=xT[128 * t:128 * (t + 1), 0:512])
                  else:
                      for h in range(2):
                          eng = nc.sync if h == 0 else nc.scalar
                          ts_ = slice(4 * h, 4 * h + 4)
                          eng.dma_start(out=xt[:, ts_, 512 * c:512 * (c + 1)],
                                        in_=xT_r[:, ts_, 512 * c:512 * (c + 1)])
                  if c == 1:
                      # pair-0 weights arrive right after xt chunk 2, just in
                      # time for the dense QK0 phase that follows V-proj
                      wq0, wk0, qt0, kt0 = load_qk_weights(0)
              nc.gpsimd.memset(v_sb[:, :, :, DH:DH + 1], 1.0)

              # ---- PE work units (each closure emits ONE matmul; the
              # group-final one also emits the PSUM->SBUF evacuation) ----
              def v_units_for_chunk(c4):
                  units = []
                  for kt in range(4 * c4, 4 * c4 + 4):
                      st = {}
                      for t in range(n_dt):
                          def u(t=t, st=st, kt=kt):
                              if t == 0:
                                  st['ps'] = ps.tile([128, dl], F32,
                                                     name=f"{R}pv_{kt}",
                                                     tag="pp", bufs=2)
                              nc.tensor.matmul(st['ps'],
                                               xt[:, t, 128 * kt:128 * (kt + 1)],
                                               wv_sb[:, t, :],
                                               start=(t == 0), stop=(t == n_dt - 1))
                              if t == n_dt - 1:
                                  nc.vector.tensor_copy(
                                      out=v_sb[:, kt, :, 0:DH],
                                      in_=st['ps'].rearrange("p (h e) -> p h e", e=DH))
                          units.append(u)
                  return units

              def qk_units_for_chunk(p, wq_sb, wk_sb, qt, kt_sb, c4):
                  units = []
                  for wi, (w_sb, dst) in enumerate(((wq_sb, qt), (wk_sb, kt_sb))):
                      st = {}
                      for t in range(n_dt):
                          def u(t=t, st=st, wi=wi, w_sb=w_sb, dst=dst, c4=c4):
                              if t == 0:
                                  st['ps'] = ps.tile([128, 512], F32,
                                                     name=f"{R}fq{p}_{c4}_{wi}",
                                                     tag="pp", bufs=2)
                              nc.tensor.matmul(st['ps'], w_sb[:, t, :],
                                               xt[:, t, 512 * c4:512 * (c4 + 1)],
                                               start=(t == 0), stop=(t == n_dt - 1))
                              if t == n_dt - 1:
                                  nc.vector.tensor_copy(
                                      out=dst[:, 512 * c4:512 * (c4 + 1)],
                                      in_=st['ps'])
                          units.append(u)
                  return units

              def o_units_for_stile(st_i, wo_sb):
                  units = []
                  st = {}
                  for oc in range(n_oc):
                      for ct in range(pairs):
                          def u(oc=oc, ct=ct, st=st, st_i=st_i):
                              if oc == 0 and ct == 0:
                                  st['ot'] = sp.tile([128, d], F32,
                                                     name=f"{R}ot_{st_i}",
                                                     tag="ot", bufs=4)
                              if ct == 0:
                                  st['ps'] = ps.tile([128, 512], F32,
                                                     name=f"{R}po_{st_i}_{oc}",
                                                     tag="pp", bufs=2)
                              nc.tensor.matmul(
                                  st['ps'],
                                  ctx_all[:, ct, 128 * st_i:128 * (st_i + 1)],
                                  wo_sb[:, ct, 512 * oc:512 * (oc + 1)],
                                  start=(ct == 0), stop=(ct == pairs - 1))
                              if ct == pairs - 1:
                                  nc.vector.tensor_copy(
                                      out=st['ot'][:, 512 * oc:512 * (oc + 1)],
                                      in_=st['ps'])
                                  if oc == n_oc - 1:
                                      oeng = nc.sync if st_i % 2 == 0 else nc.scalar
                                      oeng.dma_start(
                                          out=out[128 * st_i:128 * (st_i + 1), :],
                                          in_=st['ot'])
                          units.append(u)
                  return units

              # ---- dense V projection, then dense pair-0 Q/K projection ----
              for c4 in range(n_ch):
                  for u in v_units_for_chunk(c4):
                      u()
              for c4 in range(n_ch):
                  for u in qk_units_for_chunk(0, wq0, wk0, qt0, kt0, c4):
                      u()

              # shared filler pool with forced-drain support
              pool = {'units': [], 'ui': 0}

              def fill(k):
                  for _ in range(k):
                      if pool['ui'] < len(pool['units']):
                          pool['units'][pool['ui']](); pool['ui'] += 1

              def drain(upto=None):
                  stop = len(pool['units']) if upto is None else upto
                  while pool['ui'] < stop:
                      pool['units'][pool['ui']](); pool['ui'] += 1

              def attn_chunk(p, c4, qt, kt_sb):
                  q0 = 512 * c4
                  ctxA = ps.tile([DH + 1, 512], F32, name=f"{R}cA_{p}_{c4}",
                                 tag="ctxA", bufs=1)
                  ctxB = ps.tile([DH + 1, 512], F32, name=f"{R}cB_{p}_{c4}",
                                 tag="ctxB", bufs=1)
                  nj = 4 * c4 + 4
                  pending = None
                  for j in range(nj):
                      m = j - 4 * c4
                      n0 = 128 * m if m >= 0 else 0
                      ks = slice(128 * j, 128 * (j + 1))
                      stAB = ps.tile([128, 2, 512], F32,
                                     name=f"{R}s_{p}_{c4}_{j}",
                                     tag="stAB", bufs=2)
                      qs = slice(q0 + n0, q0 + 512)
                      nc.tensor.matmul(stAB[:, 0, n0:512], kt_sb[0:64, ks],
                                       qt[0:64, qs], start=True, stop=True)
                      nc.tensor.matmul(stAB[:, 1, n0:512], kt_sb[64:128, ks],
                                       qt[64:128, qs], start=True, stop=True)
                      pt = sp.tile([128, 2, 512], BF16,
                                   name=f"{R}pt_{p}_{c4}_{j}",
                                   tag="pt", bufs=5)
                      nc.scalar.activation(out=pt[:, :, n0:512],
                                           in_=stAB[:, :, n0:512],
                                           func=AFT.Exp, scale=0.125)
                      if m >= 0:
                          nc.vector.tensor_mul(pt[:, 0, n0:n0 + 128],
                                               pt[:, 0, n0:n0 + 128], tri_sb)
                          nc.vector.tensor_mul(pt[:, 1, n0:n0 + 128],
                                               pt[:, 1, n0:n0 + 128], tri_sb)
                      fill(2)
                      if pending is not None:
                          _emit_pv(nc, v_sb, ctxA, ctxB, p, pending, nj)
                      pending = (j, pt, n0)
                  _emit_pv(nc, v_sb, ctxA, ctxB, p, pending, nj)

                  # evacuate fast to an f32 staging tile (frees the ctx
                  # banks), then normalize into bf16 ctx_all off the
                  # critical path
                  dn = sp.tile([1, 2, 512], F32, name=f"{R}dn_{p}_{c4}",
                               tag="dn", bufs=2)
                  nc.vector.tensor_copy(out=dn[:, 0, :], in_=ctxA[DH:DH + 1, :])
                  nc.vector.tensor_copy(out=dn[:, 1, :], in_=ctxB[DH:DH + 1, :])
                  cs = sp.tile([128, 2, 512], F32, name=f"{R}cs_{p}_{c4}",
                               tag="cstg", bufs=3)
                  nc.vector.tensor_copy(out=cs[0:64, 0, :], in_=ctxA[0:DH, :])
                  nc.vector.tensor_copy(out=cs[64:128, 1, :], in_=ctxB[0:DH, :])
                  rc = sp.tile([1, 2, 512], F32, name=f"{R}rc_{p}_{c4}",
                               tag="rc", bufs=2)
                  nc.vector.reciprocal_approx_fast(out=rc, in_=dn)
                  rb = sp.tile([128, 2, 512], F32, name=f"{R}rb_{p}_{c4}",
                               tag="rb", bufs=2)
                  nc.gpsimd.partition_broadcast(rb, rc)
                  nc.vector.tensor_mul(ctx_all[0:64, p, q0:q0 + 512],
                                       cs[0:64, 0, :], rb[0:64, 0, :])
                  nc.vector.tensor_mul(ctx_all[64:128, p, q0:q0 + 512],
                                       cs[64:128, 1, :], rb[64:128, 1, :])

              # ---- pairs 0,1: pair-major, fillers = next pair's Q/K ----
              wq1, wk1, qt1, kt1 = load_qk_weights(1)
              for cc in range(n_ch):
                  pool['units'] += qk_units_for_chunk(1, wq1, wk1, qt1, kt1, cc)
              for c4 in range(n_ch):
                  attn_chunk(0, c4, qt0, kt0)
              drain()

              wq2, wk2, qt2, kt2 = load_qk_weights(2)
              wq3, wk3, qt3, kt3 = load_qk_weights(3)
              pool = {'units': [], 'ui': 0}
              for cc in range(n_ch):
                  pool['units'] += qk_units_for_chunk(2, wq2, wk2, qt2, kt2, cc)
              for c4 in range(n_ch):
                  attn_chunk(1, c4, qt1, kt1)
              drain()

              # ---- pairs 2,3: chunk-interleaved; fillers = pair-3 Q/K and
              # the O-projection of already-normalized chunks ----
              wo_sb = sp.tile([128, pairs, d], BF16, name=f"{R}wo_sb", tag="wvo")
              for ct in range(pairs):
                  nc.sync.dma_start(out=wo_sb[:, ct, :],
                                    in_=woT[128 * ct:128 * (ct + 1), :])
              pool = {'units': [], 'ui': 0}
              qk3_end = {}
              for cc in range(n_ch):
                  pool['units'] += qk_units_for_chunk(3, wq3, wk3, qt3, kt3, cc)
                  qk3_end[cc] = len(pool['units'])
              for c4 in range(n_ch):
                  if c4 > 0:
                      for st_i in range(4 * (c4 - 1), 4 * c4):
                          pool['units'] += o_units_for_stile(st_i, wo_sb)
                  attn_chunk(2, c4, qt2, kt2)
                  drain(qk3_end[c4])   # pair-3 needs its q/k for this chunk
                  attn_chunk(3, c4, qt3, kt3)
              drain()

              # O-projection remainder: the last chunk's s-tiles
              for st_i in range(4 * (n_ch - 1), n_kt):
                  for u in o_units_for_stile(st_i, wo_sb):
                      u()

    nc.compile()
    return nc


def _emit_pv(nc, v_sb, ctxA, ctxB, p, pending, nj):
    j, pt, n0 = pending
    start = (j == 0)
    stop = (j == nj - 1)
    nc.tensor.matmul(ctxA[:, n0:512], v_sb[:, j, 2 * p, :], pt[:, 0, n0:512],
                     start=start, stop=stop)
    nc.tensor.matmul(ctxB[:, n0:512], v_sb[:, j, 2 * p + 1, :], pt[:, 1, n0:512],
                     start=start, stop=stop)


def make_tri():
    import ml_dtypes
    k = np.arange(128)[:, None]
    q = np.arange(128)[None, :]
    return (k <= q).astype(ml_dtypes.bfloat16)


def shard_inputs(in_features, q_weight, k_weight, v_weight, o_weight):
    """-> list of 8 per-core input dicts (matmul operands as bf16)."""
    import ml_dtypes
    bf = ml_dtypes.bfloat16
    tri = make_tri()
    maps = []
    for c in range(N_CORES):
        b, g = divmod(c, 2)
        hs = slice(DL * g, DL * (g + 1))   # local head dims in the full D
        maps.append({
            "xT": np.ascontiguousarray(in_features[b].T).astype(bf),
            "wqT": np.ascontiguousarray(q_weight[hs, :].T).astype(bf),
            "wkT": np.ascontiguousarray(k_weight[hs, :].T).astype(bf),
            "wvT": np.ascontiguousarray(v_weight[hs, :].T).astype(bf),
            "woT": np.ascontiguousarray(o_weight[:, hs].T).astype(bf),
            "tri": tri,
        })
    return maps


def gather_output(results):
    """results: list of 8 dicts with 'out' [S, D] partials -> [B, S, D]."""
    return np.stack([results[2 * b]["out"] + results[2 * b + 1]["out"]
                     for b in range(B)])


_nc_cache = {}


def kernel(in_features, q_weight, k_weight, v_weight, o_weight):
    from concourse.bass_utils import run_bass_kernel_spmd
    if "nc" not in _nc_cache:
        _nc_cache["nc"] = build_nc()
    nc = _nc_cache["nc"]
    in_maps = shard_inputs(np.asarray(in_features, dtype=np.float32),
                           np.asarray(q_weight, dtype=np.float32),
                           np.asarray(k_weight, dtype=np.float32),
                           np.asarray(v_weight, dtype=np.float32),
                           np.asarray(o_weight, dtype=np.float32))
    res = run_bass_kernel_spmd(nc, in_maps, core_ids=list(range(N_CORES)))
    return gather_output(res.results)
